# revision 1
# baseline (speedup 1.0000x reference)
"""Multi-head attention kernel for Trainium2, tensor-parallel over heads on 8 cores.

Strategy (per core c, heads [2c, 2c+1]):
  - host feeds X^T [D, B*S] (shared), per-core transposed head weights, and the
    matching Wo column-slice; each core computes a full-shape partial of the
    output projection, host sums the 8 partials and adds bo.
  - on device everything is computed in "transposed" orientation so every
    matmul contracts over the partition dim with no on-device transposes of
    activations (only V needs a PE-transpose). All matmul operands are fp16
    (1 cyc/row on the PE; ~5e-4 final rel err), accumulation stays fp32:
      QT/KT/VT [e, s] = W @ X^T          (fp16 matmuls, N=512)
      S^T [t, s]      = KT.T @ QT        (per (b, head), C=64, head pair
                                          row-group-packed and concurrent)
      P^T             = exp(S^T / 8)     (ACT, PSUM->SBUF, the wall: ~284us)
      [avT ; l]       = [V | 1].T @ P^T  (fused unnormalized attention + sum)
      Z               = avT * (1/l)      (reciprocal_approx_fast + DRAM-bounce
                                          partition broadcast)
      out_partial     = Z.T @ WoT_slice  (PSUM -> SBUF -> DRAM)
"""

import numpy as np

import concourse.bass as bass
import concourse.mybir as mybir
import concourse.tile as tile
from concourse import bacc
from concourse.bass_utils import run_bass_kernel_spmd
from concourse.masks import make_identity

# Problem shapes (hardcoded per contract).
B, S, D = 4, 2048, 1024
H, E = 16, 64
NCORES = 8
HPC = H // NCORES          # heads per core = 2
EC = HPC * E               # per-core head width = 128
BS = B * S                 # 8192 rows
P = 128
DC = D // P                # 8 contraction chunks for the projections
ST = 512                   # s tile (matmul moving free dim)
N_ST = S // ST             # 4 s-tiles per batch
TCH = S // P               # 16 key chunks per batch

F32 = mybir.dt.float32
F32R = mybir.dt.float32r
F16 = mybir.dt.float16
EXP = mybir.ActivationFunctionType.Exp


def _r(ap):
    return ap.bitcast(F32R)


def build_module():
    """Build the single-core Bass module (same NEFF runs SPMD on all 8 cores)."""
    from contextlib import ExitStack

    nc = bacc.Bacc("TRN2", target_bir_lowering=False, debug=False)
    xt = nc.dram_tensor("xt", [D, BS], F16, kind="ExternalInput").ap()
    wq = nc.dram_tensor("wq_t", [D, EC], F16, kind="ExternalInput").ap()
    wk = nc.dram_tensor("wk_t", [D, EC], F16, kind="ExternalInput").ap()
    wv = nc.dram_tensor("wv_t", [D, EC], F16, kind="ExternalInput").ap()
    bq = nc.dram_tensor("bq", [EC, 1], F32, kind="ExternalInput").ap()
    bk = nc.dram_tensor("bk", [EC, 1], F32, kind="ExternalInput").ap()
    bv = nc.dram_tensor("bv", [EC, 1], F32, kind="ExternalInput").ap()
    wo = nc.dram_tensor("wo_t", [EC, D], F16, kind="ExternalInput").ap()
    outp = nc.dram_tensor("out_p", [BS, D], F32, kind="ExternalOutput").ap()

    xt_r = xt.rearrange("(dc p) s -> p dc s", p=P)    # [128, 8, 8192]
    wq_r = wq.rearrange("(dc p) e -> p dc e", p=P)    # [128, 8, 128]
    wk_r = wk.rearrange("(dc p) e -> p dc e", p=P)
    wv_r = wv.rearrange("(dc p) e -> p dc e", p=P)

    with tile.TileContext(nc) as tc, ExitStack() as ctx:
        singles = ctx.enter_context(tc.tile_pool(name="singles", bufs=1))

        wq_sb = singles.tile([P, DC, EC], F16, tag="wq")
        wk_sb = singles.tile([P, DC, EC], F16, tag="wk")
        wv_sb = singles.tile([P, DC, EC], F16, tag="wv")
        nc.sync.dma_start(wq_sb[:], wq_r)
        nc.sync.dma_start(wk_sb[:], wk_r)
        nc.sync.dma_start(wv_sb[:], wv_r)
        bq_sb = singles.tile([EC, 1], F32, tag="bq")
        bk_sb = singles.tile([EC, 1], F32, tag="bk")
        bv_sb = singles.tile([EC, 1], F32, tag="bv")
        nc.sync.dma_start(bq_sb[:], bq)
        nc.sync.dma_start(bk_sb[:], bk)
        nc.sync.dma_start(bv_sb[:], bv)
        wo_sb = singles.tile([EC, D], F16, tag="wo")
        nc.sync.dma_start(wo_sb[:], wo)
        ident = singles.tile([P, P], F32, tag="ident")
        make_identity(nc, ident[:])

        # Per-batch persistent activations: [e, s] projections and V_ext.
        qt = [singles.tile([EC, S], F16, tag=f"qt{b}", name=f"qt{b}") for b in range(B)]
        kt = [singles.tile([EC, S], F16, tag=f"kt{b}", name=f"kt{b}") for b in range(B)]
        vt = [singles.tile([EC, S], F32, tag=f"vtz{b}", name=f"vt{b}") for b in range(B)]
        # V_ext layout: [t-part, t-chunk, 130] = [V_h0 | 1 | V_h1 | 1]
        vx = [singles.tile([P, TCH, 2 * E + 2], F16, tag=f"vx{b}", name=f"vx{b}") for b in range(B)]
        for b in range(B):
            nc.vector.memset(vx[b][:, :, E : E + 1], 1.0)
            nc.vector.memset(vx[b][:, :, 2 * E + 1 : 2 * E + 2], 1.0)

        # Pipelined per-batch schedule: proj(0); attn(b) overlapped with
        # proj(b+1). One PSUM pool, 8 banks total:
        #   "mm" (proj-accum / transpose / out-proj) 2, "sc" 4, "av" 2.
        z = [singles.tile([EC, S], F16, tag=f"z{b}", name=f"z{b}") for b in range(B)]
        with (
            tc.tile_pool(name="xload", bufs=4) as xpool,
            tc.tile_pool(name="pexp", bufs=6) as ppool,
            tc.tile_pool(name="bcast", bufs=3) as bpool,
            tc.tile_pool(name="ostage", bufs=4) as opool,
            tc.tile_pool(name="lrow", bufs=2, space="DRAM") as dpool,
            tc.tile_pool(name="psum", bufs=2, space="PSUM") as psum,
            tc.tile_pool(name="psum_av", bufs=1, space="PSUM") as psum_av,
        ):

            def emit_proj(b, st, lazy=True):
                # Lower scheduling priority so overlapped projection work only
                # fills PE gaps instead of starving the scores->exp chain.
                from contextlib import nullcontext
                prio = tc.high_priority(offset=-1000000) if lazy else nullcontext()
                with prio:
                    g = b * N_ST + st
                    sl = slice(st * ST, (st + 1) * ST)
                    x_t = xpool.tile([P, DC, ST], F16, tag="xt", name="x_t")
                    nc.sync.dma_start(x_t[:], xt_r[:, :, g * ST : (g + 1) * ST])
                    for w_sb, b_sb, dst in (
                        (wq_sb, bq_sb, qt[b]),
                        (wk_sb, bk_sb, kt[b]),
                        (wv_sb, bv_sb, vt[b]),
                    ):
                        ps = psum.tile([P, ST], F32, tag="mm", name="ps")
                        for dc in range(DC):
                            nc.tensor.matmul(
                                ps[:], w_sb[:, dc], x_t[:, dc],
                                start=(dc == 0), stop=(dc == DC - 1),
                            )
                        nc.vector.tensor_scalar_add(dst[:, sl], ps[:], b_sb[:])

            def emit_trans(b, lazy=True, chunks=None):
                # PE-transpose batch b's V chunks into V_ext (gap-filler work).
                from contextlib import nullcontext
                prio = tc.high_priority(offset=-1000000) if lazy else nullcontext()
                with prio:
                    for tch in (range(TCH) if chunks is None else chunks):
                        tp = psum.tile([P, ST], F32, tag="mm", name="tp")
                        nc.tensor.transpose(
                            tp[:, 0:P], vt[b][:, tch * P : (tch + 1) * P], ident[:]
                        )
                        nc.vector.tensor_copy(vx[b][:, tch, 0:E], tp[:, 0:E])
                        nc.vector.tensor_copy(
                            vx[b][:, tch, E + 1 : 2 * E + 1], tp[:, E : 2 * E]
                        )

            def emit_attn(b, st):
                if True:
                    ssl = slice(st * ST, (st + 1) * ST)
                    av = psum_av.tile([P, 2, ST], F32, tag="av", name="av")
                    av0 = av[:, 0]
                    av1 = av[:, 1]
                    for t in range(TCH):
                        tsl = slice(t * P, (t + 1) * P)
                        sc = psum.tile([P, 2, ST], F32, tag="sc", name="sc")
                        nc.tensor.matmul(
                            sc[:, 0], kt[b][0:E, tsl], qt[b][0:E, ssl],
                            start=True, stop=True,
                        )
                        nc.tensor.matmul(
                            sc[:, 1], kt[b][E : 2 * E, tsl], qt[b][E : 2 * E, ssl],
                            start=True, stop=True,
                        )
                        pt = ppool.tile([P, 2, ST], F16, tag="pt", name="pt")
                        nc.scalar.activation(pt[:], sc[:], EXP, scale=0.125)
                        nc.tensor.matmul(
                            av0[0 : E + 1], vx[b][:, t, 0 : E + 1], pt[:, 0],
                            start=(t == 0), stop=(t == TCH - 1),
                        )
                        nc.tensor.matmul(
                            av1[0 : E + 1], vx[b][:, t, E + 1 : 2 * E + 2], pt[:, 1],
                            start=(t == 0), stop=(t == TCH - 1),
                        )
                    # Unnormalized copy out of PSUM (frees the av banks fast),
                    # reciprocal of the fused row-sums, broadcast via DRAM.
                    with tc.high_priority():
                        nc.vector.tensor_copy(z[b][0:E, ssl], av0[0:E])
                        nc.vector.tensor_copy(z[b][E : 2 * E, ssl], av1[0:E])
                        lr = bpool.tile([1, 2, ST], F32, tag="lr", name="lr")
                        # custom DVE ops only work at partition base 0: plain-copy
                        # both PSUM l-rows down in one op, then reciprocal in place.
                        nc.vector.tensor_copy(lr[0:1], av[E : E + 1, :, :])
                        nc.vector.reciprocal_approx_fast(out=lr[0:1], in_=lr[0:1])
                    lrow = dpool.tile([2, ST], F32, tag="lrow", name="lrow")
                    nc.sync.dma_start(
                        bass.AP(tensor=lrow.tensor, offset=lrow.offset,
                                ap=[[0, 1]] + list(lrow.ap)),
                        lr[0:1, :, :],
                    )
                    bc = bpool.tile([P, ST], F32, tag="bc", name="bc")
                    nc.sync.dma_start(
                        bc[0:E],
                        bass.AP(tensor=lrow.tensor, offset=lrow.offset,
                                ap=[[0, E]] + list(lrow[0, :].ap)),
                    )
                    nc.sync.dma_start(
                        bc[E : 2 * E],
                        bass.AP(tensor=lrow.tensor, offset=lrow.offset + ST,
                                ap=[[0, E]] + list(lrow[1, :].ap)),
                    )
                    nc.vector.tensor_mul(z[b][0:E, ssl], z[b][0:E, ssl], bc[0:E])
                    nc.vector.tensor_mul(
                        z[b][E : 2 * E, ssl], z[b][E : 2 * E, ssl], bc[E : 2 * E]
                    )
                    # Output projection for this s-tile's four 128-row chunks.
                    for c in range(ST // P):
                        zsl = slice(st * ST + c * P, st * ST + (c + 1) * P)
                        rows = slice(b * S + st * ST + c * P, b * S + st * ST + (c + 1) * P)
                        for oh in range(D // 512):
                            po = psum.tile([P, ST], F32, tag="mm", name="po")
                            nc.tensor.matmul(
                                po[:], z[b][:, zsl], wo_sb[:, oh * 512 : (oh + 1) * 512],
                                start=True, stop=True,
                            )
                            osb = opool.tile([P, 512], F32, tag="osb", name="osb")
                            nc.vector.tensor_copy(osb[:], po[:])
                            nc.sync.dma_start(outp[rows, oh * 512 : (oh + 1) * 512], osb[:])

            for st in range(N_ST):
                emit_proj(0, st, lazy=False)
                emit_trans(0, lazy=False, chunks=range(st * 4, (st + 1) * 4))
            for b in range(1, B):
                for st in range(N_ST):
                    emit_proj(b, st, lazy=False)
            for b in range(B):
                if b + 1 < B:
                    emit_trans(b + 1, lazy=True)
                for st in range(N_ST):
                    emit_attn(b, st)
    nc.finalize()
    return nc


_NC_CACHE = None


def _get_module():
    global _NC_CACHE
    if _NC_CACHE is None:
        _NC_CACHE = build_module()
    return _NC_CACHE


def prepare_in_maps(inputs):
    x = np.ascontiguousarray(np.asarray(inputs["input_matrix"], np.float32))
    wq = np.asarray(inputs["Wq"], np.float32)
    wk = np.asarray(inputs["Wk"], np.float32)
    wv = np.asarray(inputs["Wv"], np.float32)
    bq = np.asarray(inputs["bq"], np.float32)
    bk = np.asarray(inputs["bk"], np.float32)
    bv = np.asarray(inputs["bv"], np.float32)
    wo = np.asarray(inputs["Wo"], np.float32)

    xt = np.ascontiguousarray(x.reshape(BS, D).T.astype(np.float16))  # [D, BS]
    in_maps = []
    for c in range(NCORES):
        hs = slice(HPC * c, HPC * (c + 1))
        m = {
            "xt": xt,
            "wq_t": np.ascontiguousarray(wq[hs].transpose(2, 0, 1).reshape(D, EC).astype(np.float16)),
            "wk_t": np.ascontiguousarray(wk[hs].transpose(2, 0, 1).reshape(D, EC).astype(np.float16)),
            "wv_t": np.ascontiguousarray(wv[hs].transpose(2, 0, 1).reshape(D, EC).astype(np.float16)),
            "bq": np.ascontiguousarray(bq[hs].reshape(EC, 1)),
            "bk": np.ascontiguousarray(bk[hs].reshape(EC, 1)),
            "bv": np.ascontiguousarray(bv[hs].reshape(EC, 1)),
            "wo_t": np.ascontiguousarray(wo[:, EC * c : EC * (c + 1)].T.astype(np.float16)),
        }
        in_maps.append(m)
    return in_maps


def finish(results, inputs):
    bo = np.asarray(inputs["bo"], np.float32)
    acc = results[0]["out_p"].astype(np.float64)
    for r in results[1:]:
        acc += r["out_p"]
    out = (acc + bo).astype(np.float32)
    return out.reshape(B, S, D)


def kernel(**inputs):
    nc = _get_module()
    in_maps = prepare_in_maps(inputs)
    res = run_bass_kernel_spmd(nc, in_maps, core_ids=list(range(NCORES)))
    return finish(res.results, inputs)


if __name__ == "__main__":
    import reference

    inputs = {k: np.asarray(v) for k, v in reference.setup_inputs().items()}
    out = kernel(**inputs)
    print(out.shape, out.dtype)



# revision 4
# speedup vs baseline: 1.0466x; 1.0466x over previous
"""Multi-head attention kernel for Trainium2, tensor-parallel over heads on 8 cores.

Strategy (per core c, heads [2c, 2c+1]):
  - host feeds X^T [D, B*S] (shared), per-core transposed head weights, and the
    matching Wo column-slice; each core computes a full-shape partial of the
    output projection (fp16), host sums the 8 partials and adds bo.
  - on device everything is computed in "transposed" orientation so every
    matmul contracts over the partition dim with no on-device transposes of
    activations (only V needs a PE-transpose). All matmul operands are fp16
    (1 cyc/row on the PE; ~5e-4 final rel err), accumulation stays fp32:
      QT/KT/VT [e, s] = W @ X^T          (fp16 matmuls, N=512)
      S^T [t, s]      = KT.T @ QT        (per (b, head), C=64)
      P^T             = exp(S^T / 8)     (ACT, PSUM->SBUF)
      [avT ; l]       = [V | 1].T @ P^T  (fused unnormalized attention + sum)
      Z               = avT * (1/l)      (reciprocal_approx_fast + DRAM-bounce
                                          partition broadcast)
      out_partial     = Z.T @ WoT_slice  (PSUM -> SBUF fp16 -> DRAM)

  The emission order software-pipelines the whole kernel: the PE queue for
  attention group (b, st) has "filler" units (next batch's projections /
  V transposes / the previous group's output projection) interleaved between
  score/AV chunk matmuls, so the tensor engine never drains while the ACT
  engine chews through the exp wall (~284us) and vice versa.
"""

import numpy as np

import concourse.bass as bass
import concourse.mybir as mybir
import concourse.tile as tile
from concourse import bacc
from concourse.bass_utils import run_bass_kernel_spmd
from concourse.masks import make_identity

# Problem shapes (hardcoded per contract).
B, S, D = 4, 2048, 1024
H, E = 16, 64
NCORES = 8
HPC = H // NCORES          # heads per core = 2
EC = HPC * E               # per-core head width = 128
BS = B * S                 # 8192 rows
P = 128
DC = D // P                # 8 contraction chunks for the projections
ST = 512                   # s tile (matmul moving free dim)
N_ST = S // ST             # 4 s-tiles per batch
TCH = S // P               # 16 key chunks per batch

F32 = mybir.dt.float32
F16 = mybir.dt.float16
EXP = mybir.ActivationFunctionType.Exp


def build_module():
    """Build the single-core Bass module (same NEFF runs SPMD on all 8 cores)."""
    from contextlib import ExitStack

    nc = bacc.Bacc("TRN2", target_bir_lowering=False, debug=False)
    xt = nc.dram_tensor("xt", [D, BS], F16, kind="ExternalInput").ap()
    wq = nc.dram_tensor("wq_t", [D, EC], F16, kind="ExternalInput").ap()
    wk = nc.dram_tensor("wk_t", [D, EC], F16, kind="ExternalInput").ap()
    wv = nc.dram_tensor("wv_t", [D, EC], F16, kind="ExternalInput").ap()
    bq = nc.dram_tensor("bq", [EC, 1], F32, kind="ExternalInput").ap()
    bk = nc.dram_tensor("bk", [EC, 1], F32, kind="ExternalInput").ap()
    bv = nc.dram_tensor("bv", [EC, 1], F32, kind="ExternalInput").ap()
    wo = nc.dram_tensor("wo_t", [EC, D], F16, kind="ExternalInput").ap()
    outp = nc.dram_tensor("out_p", [BS, D], F16, kind="ExternalOutput").ap()

    xt_r = xt.rearrange("(dc p) s -> p dc s", p=P)    # [128, 8, 8192]
    wq_r = wq.rearrange("(dc p) e -> p dc e", p=P)    # [128, 8, 128]
    wk_r = wk.rearrange("(dc p) e -> p dc e", p=P)
    wv_r = wv.rearrange("(dc p) e -> p dc e", p=P)

    with tile.TileContext(nc) as tc, ExitStack() as ctx:
        singles = ctx.enter_context(tc.tile_pool(name="singles", bufs=1))

        wq_sb = singles.tile([P, DC, EC], F16, tag="wq")
        wk_sb = singles.tile([P, DC, EC], F16, tag="wk")
        wv_sb = singles.tile([P, DC, EC], F16, tag="wv")
        nc.sync.dma_start(wq_sb[:], wq_r)
        nc.sync.dma_start(wk_sb[:], wk_r)
        nc.sync.dma_start(wv_sb[:], wv_r)
        bq_sb = singles.tile([EC, 1], F32, tag="bq")
        bk_sb = singles.tile([EC, 1], F32, tag="bk")
        bv_sb = singles.tile([EC, 1], F32, tag="bv")
        nc.sync.dma_start(bq_sb[:], bq)
        nc.sync.dma_start(bk_sb[:], bk)
        nc.sync.dma_start(bv_sb[:], bv)
        wo_sb = singles.tile([EC, D], F16, tag="wo")
        nc.sync.dma_start(wo_sb[:], wo)
        ident = singles.tile([P, P], F32, tag="ident")
        make_identity(nc, ident[:])

        # Per-batch persistent activations: [e, s] projections and V_ext.
        qt = [singles.tile([EC, S], F16, tag=f"qt{b}", name=f"qt{b}") for b in range(B)]
        kt = [singles.tile([EC, S], F16, tag=f"kt{b}", name=f"kt{b}") for b in range(B)]
        vt = [singles.tile([EC, S], F32, tag=f"vtz{b}", name=f"vt{b}") for b in range(B)]
        # V_ext layout: [t-part, t-chunk, 130] = [V_h0 | 1 | V_h1 | 1]
        vx = [singles.tile([P, TCH, 2 * E + 2], F16, tag=f"vx{b}", name=f"vx{b}") for b in range(B)]
        for b in range(B):
            nc.vector.memset(vx[b][:, :, E : E + 1], 1.0)
            nc.vector.memset(vx[b][:, :, 2 * E + 1 : 2 * E + 2], 1.0)

        z = [singles.tile([EC, S], F16, tag=f"z{b}", name=f"z{b}") for b in range(B)]
        with (
            tc.tile_pool(name="xload", bufs=4) as xpool,
            tc.tile_pool(name="pexp", bufs=6) as ppool,
            tc.tile_pool(name="bcast", bufs=3) as bpool,
            tc.tile_pool(name="ostage", bufs=4) as opool,
            tc.tile_pool(name="lrow", bufs=2, space="DRAM") as dpool,
            tc.tile_pool(name="psum", bufs=2, space="PSUM") as psum,
            tc.tile_pool(name="psum_av", bufs=1, space="PSUM") as psum_av,
        ):
            xtiles = {}

            def emit_xload(b, st):
                g = b * N_ST + st
                x_t = xpool.tile([P, DC, ST], F16, tag="xt", name=f"x{b}_{st}")
                nc.sync.dma_start(x_t[:], xt_r[:, :, g * ST : (g + 1) * ST])
                xtiles[(b, st)] = x_t

            def emit_proj(b, st, kind):
                w_sb, b_sb, dst = {
                    "q": (wq_sb, bq_sb, qt[b]),
                    "k": (wk_sb, bk_sb, kt[b]),
                    "v": (wv_sb, bv_sb, vt[b]),
                }[kind]
                x_t = xtiles[(b, st)]
                sl = slice(st * ST, (st + 1) * ST)
                ps = psum.tile([P, ST], F32, tag="mm", name="ps")
                for dc in range(DC):
                    nc.tensor.matmul(
                        ps[:], w_sb[:, dc], x_t[:, dc],
                        start=(dc == 0), stop=(dc == DC - 1),
                    )
                nc.vector.tensor_scalar_add(dst[:, sl], ps[:], b_sb[:])

            def emit_trans4(b, st):
                # PE-transpose 4 of batch b's V chunks into V_ext.
                for tch in range(st * 4, st * 4 + 4):
                    tp = psum.tile([P, ST], F32, tag="mm", name="tp")
                    nc.tensor.transpose(
                        tp[:, 0:P], vt[b][:, tch * P : (tch + 1) * P], ident[:]
                    )
                    nc.vector.tensor_copy(vx[b][:, tch, 0:E], tp[:, 0:E])
                    nc.vector.tensor_copy(
                        vx[b][:, tch, E + 1 : 2 * E + 1], tp[:, E : 2 * E]
                    )

            def emit_outproj(b, st):
                # Output projection for this s-tile's four 128-row chunks.
                for c in range(ST // P):
                    zsl = slice(st * ST + c * P, st * ST + (c + 1) * P)
                    rows = slice(b * S + st * ST + c * P, b * S + st * ST + (c + 1) * P)
                    for oh in range(D // 512):
                        po = psum.tile([P, ST], F32, tag="mm", name="po")
                        nc.tensor.matmul(
                            po[:], z[b][:, zsl], wo_sb[:, oh * 512 : (oh + 1) * 512],
                            start=True, stop=True,
                        )
                        osb = opool.tile([P, 512], F16, tag="osb", name="osb")
                        nc.vector.tensor_copy(osb[:], po[:])
                        nc.sync.dma_start(outp[rows, oh * 512 : (oh + 1) * 512], osb[:])

            def emit_attn_group(b, st, fillers):
                ssl = slice(st * ST, (st + 1) * ST)
                av = psum_av.tile([P, 2, ST], F32, tag="av", name="av")
                av0 = av[:, 0]
                av1 = av[:, 1]
                fq = list(fillers)
                for t in range(TCH):
                    tsl = slice(t * P, (t + 1) * P)
                    sc = psum.tile([P, 2, ST], F32, tag="sc", name="sc")
                    nc.tensor.matmul(
                        sc[:, 0], kt[b][0:E, tsl], qt[b][0:E, ssl],
                        start=True, stop=True,
                    )
                    nc.tensor.matmul(
                        sc[:, 1], kt[b][E : 2 * E, tsl], qt[b][E : 2 * E, ssl],
                        start=True, stop=True,
                    )
                    pt = ppool.tile([P, 2, ST], F16, tag="pt", name="pt")
                    nc.scalar.activation(pt[:], sc[:], EXP, scale=0.125)
                    nc.tensor.matmul(
                        av0[0 : E + 1], vx[b][:, t, 0 : E + 1], pt[:, 0],
                        start=(t == 0), stop=(t == TCH - 1),
                    )
                    nc.tensor.matmul(
                        av1[0 : E + 1], vx[b][:, t, E + 1 : 2 * E + 2], pt[:, 1],
                        start=(t == 0), stop=(t == TCH - 1),
                    )
                    if fq:
                        fq.pop(0)()
                while fq:
                    fq.pop(0)()
                # Unnormalized copy out of PSUM (frees the av banks fast),
                # reciprocal of the fused row-sums, broadcast via DRAM.
                with tc.high_priority():
                    nc.vector.tensor_copy(z[b][0:E, ssl], av0[0:E])
                    nc.vector.tensor_copy(z[b][E : 2 * E, ssl], av1[0:E])
                    lr = bpool.tile([1, 2, ST], F32, tag="lr", name="lr")
                    # custom DVE ops only work at partition base 0: plain-copy
                    # both PSUM l-rows down in one op, then reciprocal in place.
                    nc.vector.tensor_copy(lr[0:1], av[E : E + 1, :, :])
                    nc.vector.reciprocal_approx_fast(out=lr[0:1], in_=lr[0:1])
                lrow = dpool.tile([2, ST], F32, tag="lrow", name="lrow")
                nc.sync.dma_start(
                    bass.AP(tensor=lrow.tensor, offset=lrow.offset,
                            ap=[[0, 1]] + list(lrow.ap)),
                    lr[0:1, :, :],
                )
                bc = bpool.tile([P, ST], F32, tag="bc", name="bc")
                nc.sync.dma_start(
                    bc[0:E],
                    bass.AP(tensor=lrow.tensor, offset=lrow.offset,
                            ap=[[0, E]] + list(lrow[0, :].ap)),
                )
                nc.sync.dma_start(
                    bc[E : 2 * E],
                    bass.AP(tensor=lrow.tensor, offset=lrow.offset + ST,
                            ap=[[0, E]] + list(lrow[1, :].ap)),
                )
                nc.vector.tensor_mul(z[b][0:E, ssl], z[b][0:E, ssl], bc[0:E])
                nc.vector.tensor_mul(
                    z[b][E : 2 * E, ssl], z[b][E : 2 * E, ssl], bc[E : 2 * E]
                )

            def F(fn, *a):
                return lambda: fn(*a)

            # Startup: batch 0's x + K projections + first Q tile, so the
            # first scores can issue as early as possible. V/trans of the
            # first half must precede group (0, 0)'s first AV chunks in
            # program order (Tile deps follow emission order).
            for st in range(N_ST):
                emit_xload(0, st)
            for st in range(N_ST):
                emit_proj(0, st, "k")
            emit_proj(0, 0, "q")
            emit_proj(0, 0, "v")
            emit_trans4(0, 0)
            emit_proj(0, 1, "v")
            emit_trans4(0, 1)

            # Filler units per attention group. Invariants:
            #  - Q(b, st) emitted before group (b, st) starts (prev group).
            #  - K(b+1, *) and xload(b+1, *) spread over groups (b, 1..2).
            #  - V/trans4(b+1, *) over groups (b, 3) and (b+1, 0); trans4 of
            #    the tail chunks lands within the first chunks of (b+1, 0),
            #    ahead of the AV chunks that consume them.
            #  - outproj(b, st) deferred one group past its z (bounce latency).
            fillers = {}
            fillers[(0, 0)] = [
                F(emit_proj, 0, 2, "v"), F(emit_trans4, 0, 2),
                F(emit_proj, 0, 3, "v"), F(emit_trans4, 0, 3),
                F(emit_proj, 0, 1, "q"),
            ]
            for b in range(B):
                nb = b + 1
                if b > 0:
                    fillers[(b, 0)] = [
                        F(emit_proj, b, 1, "q"),
                        F(emit_proj, b, 2, "v"), F(emit_trans4, b, 2),
                        F(emit_proj, b, 3, "v"), F(emit_trans4, b, 3),
                        F(emit_outproj, b - 1, 3),
                    ]
                if nb < B:
                    fillers[(b, 1)] = [
                        F(emit_proj, b, 2, "q"),
                        F(emit_xload, nb, 0), F(emit_xload, nb, 1),
                        F(emit_proj, nb, 0, "k"), F(emit_proj, nb, 1, "k"),
                        F(emit_outproj, b, 0),
                    ]
                    fillers[(b, 2)] = [
                        F(emit_proj, b, 3, "q"),
                        F(emit_xload, nb, 2), F(emit_xload, nb, 3),
                        F(emit_proj, nb, 2, "k"), F(emit_proj, nb, 3, "k"),
                        F(emit_outproj, b, 1),
                    ]
                    fillers[(b, 3)] = [
                        F(emit_proj, nb, 0, "v"), F(emit_trans4, nb, 0),
                        F(emit_proj, nb, 1, "v"), F(emit_trans4, nb, 1),
                        F(emit_proj, nb, 0, "q"),
                        F(emit_outproj, b, 2),
                    ]
                else:
                    fillers[(b, 1)] = [
                        F(emit_proj, b, 2, "q"), F(emit_outproj, b, 0),
                    ]
                    fillers[(b, 2)] = [
                        F(emit_proj, b, 3, "q"), F(emit_outproj, b, 1),
                    ]
                    fillers[(b, 3)] = [F(emit_outproj, b, 2)]

            for b in range(B):
                for st in range(N_ST):
                    emit_attn_group(b, st, fillers[(b, st)])
            emit_outproj(B - 1, 3)
    nc.finalize()
    return nc


_NC_CACHE = None


def _get_module():
    global _NC_CACHE
    if _NC_CACHE is None:
        _NC_CACHE = build_module()
    return _NC_CACHE


def prepare_in_maps(inputs):
    x = np.ascontiguousarray(np.asarray(inputs["input_matrix"], np.float32))
    wq = np.asarray(inputs["Wq"], np.float32)
    wk = np.asarray(inputs["Wk"], np.float32)
    wv = np.asarray(inputs["Wv"], np.float32)
    bq = np.asarray(inputs["bq"], np.float32)
    bk = np.asarray(inputs["bk"], np.float32)
    bv = np.asarray(inputs["bv"], np.float32)
    wo = np.asarray(inputs["Wo"], np.float32)

    xt = np.ascontiguousarray(x.reshape(BS, D).T.astype(np.float16))  # [D, BS]
    in_maps = []
    for c in range(NCORES):
        hs = slice(HPC * c, HPC * (c + 1))
        m = {
            "xt": xt,
            "wq_t": np.ascontiguousarray(wq[hs].transpose(2, 0, 1).reshape(D, EC).astype(np.float16)),
            "wk_t": np.ascontiguousarray(wk[hs].transpose(2, 0, 1).reshape(D, EC).astype(np.float16)),
            "wv_t": np.ascontiguousarray(wv[hs].transpose(2, 0, 1).reshape(D, EC).astype(np.float16)),
            "bq": np.ascontiguousarray(bq[hs].reshape(EC, 1)),
            "bk": np.ascontiguousarray(bk[hs].reshape(EC, 1)),
            "bv": np.ascontiguousarray(bv[hs].reshape(EC, 1)),
            "wo_t": np.ascontiguousarray(wo[:, EC * c : EC * (c + 1)].T.astype(np.float16)),
        }
        in_maps.append(m)
    return in_maps


def finish(results, inputs):
    bo = np.asarray(inputs["bo"], np.float32)
    acc = results[0]["out_p"].astype(np.float64)
    for r in results[1:]:
        acc = acc + r["out_p"].astype(np.float64)
    out = (acc + bo).astype(np.float32)
    return out.reshape(B, S, D)


def kernel(**inputs):
    nc = _get_module()
    in_maps = prepare_in_maps(inputs)
    res = run_bass_kernel_spmd(nc, in_maps, core_ids=list(range(NCORES)))
    return finish(res.results, inputs)


if __name__ == "__main__":
    import reference

    inputs = {k: np.asarray(v) for k, v in reference.setup_inputs().items()}
    out = kernel(**inputs)
    print(out.shape, out.dtype)
